# revision 25
# baseline (speedup 1.0000x reference)
"""Last-query sparse attention on 8 TRN2 NeuronCores.

Reference computation (per sample b):
    prev  = x[b, :-1, :]                 # [T-1, D]
    final = x[b, -1, :]                  # [D]
    s     = prev @ final                 # [T-1]
    w     = softmax(s)
    att   = w @ prev                     # [D]
    out   = concat(final, att)           # [2D]

Sharding: batch (B=64) split 8 ways -> 8 samples per core, no collectives.

Per-core layout: x[b] ([4096, 256] f32) is DMA'd to SBUF as [128, 32, 256]
with partition p holding rows t = p*32 + i (32KB contiguous HBM per
partition -> efficient descriptors).

Pass 1 (scores, contraction over free dim d): DVE tensor_tensor_reduce per
block i: accum[p] = sum_d x[p,i,d] * final[d] -> S[128, 32].
The self-score at t=4095 (p=127, i=31) is masked to -1e30.

Softmax: DVE row max -> GPSIMD partition_all_reduce(max) -> ACT exp with
per-partition bias (-gmax), fused row-sum accumulation; denominator via a
tiny PE matmul against a ones vector.

Pass 2 (weighted sum, contraction over t on partitions): PE matmuls
accumulating in PSUM: lhsT = exp-weights column [128, 1], rhs = x block
[128, 256] viewed as float32r (full-rate fp32 streaming), over 32 blocks.
Normalize by 1/Z on DVE, then DMA concat(final, att) to the output.
"""

import sys

sys.path.insert(0, "/opt/trn_rl_repo")

from contextlib import ExitStack

import numpy as np

import concourse.tile as tile
from concourse import bacc, mybir
from concourse.bass_utils import run_bass_kernel_spmd
from concourse.masks import make_identity

N_CORES = 8
B = 64
T = 4096
D = 256
BPC = B // N_CORES  # samples per core
P = 128
NBLK = T // P  # 32 blocks; t = p*NBLK + i
DVE_BLOCKS = 10  # pass-1 blocks handled by DVE fused ops; rest on GPSIMD
F32 = mybir.dt.float32
FP16 = mybir.dt.float16

_NC_CACHE = None


def _build():
    nc = bacc.Bacc(
        trn_type="TRN2",
        target_bir_lowering=False,
        debug=False,
        num_devices=N_CORES,
    )
    x_ext = nc.declare_dram_parameter("x", [BPC, T, D], F32, isOutput=False)
    out_ext = nc.declare_dram_parameter("out", [BPC, 2 * D], F32, isOutput=True)
    xap = x_ext.ap()
    oap = out_ext.ap()

    with ExitStack() as ctx:
        tc = ctx.enter_context(tile.TileContext(nc))
        xpool = ctx.enter_context(tc.tile_pool(name="xp", bufs=3))
        xbpool = ctx.enter_context(tc.tile_pool(name="xbp", bufs=2))
        fpool = ctx.enter_context(tc.tile_pool(name="fp", bufs=3))
        scrpool = ctx.enter_context(tc.tile_pool(name="scr", bufs=2))
        spool = ctx.enter_context(tc.tile_pool(name="sp", bufs=2))
        stat = ctx.enter_context(tc.tile_pool(name="stat", bufs=4))
        cpool = ctx.enter_context(tc.tile_pool(name="const", bufs=1))
        opool = ctx.enter_context(tc.tile_pool(name="outp", bufs=2))
        pspool = ctx.enter_context(tc.tile_pool(name="ps", bufs=2, space="PSUM"))
        statps = ctx.enter_context(tc.tile_pool(name="sps", bufs=6, space="PSUM"))

        ones = cpool.tile([P, 1], F32)
        nc.gpsimd.memset(ones[:], 1.0)
        neg_ones_row = cpool.tile([1, P], F32)
        nc.gpsimd.memset(neg_ones_row[:], -1.0)
        identity = cpool.tile([P, P], F32)
        make_identity(nc, identity[:])

        # maskbias[p] = -1e30 if p == 127 else 0 (masks the query's
        # self-score without touching a partition-127-based AP)
        pidx = cpool.tile([P, 1], mybir.dt.int32)
        nc.gpsimd.iota(pidx[:], pattern=[[0, 1]], base=0, channel_multiplier=1)
        maskbias = cpool.tile([P, 1], F32)
        nc.vector.tensor_scalar(
            out=maskbias[:],
            in0=pidx[:],
            scalar1=126,
            scalar2=None,
            op0=mybir.AluOpType.is_gt,
        )
        nc.vector.tensor_scalar_mul(maskbias[:], maskbias[:], -1.0e30)

        for b in range(BPC):
            X = xpool.tile([P, NBLK, D], F32)
            nc.sync.dma_start(X[:], xap[b].rearrange("(p i) d -> p i d", p=P))
            F = fpool.tile([P, D], F32)
            nc.sync.dma_start(F[:], xap[b, T - 1].partition_broadcast(P))

            # fp16 copy of x for pass 2: the PE streams fp16 at full rate
            # (fp32 would cost 4 cycles/row). ACT is otherwise idle.
            Xh = xbpool.tile([P, NBLK, D], FP16)
            nc.scalar.copy(Xh[:], X[:])

            # Pass 1 split: GPSIMD multiplies blocks [DVE_BLOCKS..32) into a
            # products tile in one big op, DVE reduces them segmented in one
            # op; DVE handles the first DVE_BLOCKS blocks with fused
            # multiply+reduce (scalar_tensor_tensor). Balances both engines.
            S = spool.tile([P, NBLK], F32)
            gblk = NBLK - DVE_BLOCKS
            prod = scrpool.tile([P, gblk, D], F32)
            nc.gpsimd.tensor_tensor(
                out=prod[:],
                in0=X[:, DVE_BLOCKS:NBLK, :],
                in1=F[:].unsqueeze(1).broadcast_to((P, gblk, D)),
                op=mybir.AluOpType.mult,
            )
            scr = scrpool.tile([P, D], F32)
            for i in range(DVE_BLOCKS):
                # fused multiply + free-dim reduce on DVE:
                # scr = (X*1) * F ; S[:, i] = sum(scr) in fp32
                nc.vector.scalar_tensor_tensor(
                    out=scr[:],
                    in0=X[:, i, :],
                    scalar=1.0,
                    in1=F[:],
                    op0=mybir.AluOpType.mult,
                    op1=mybir.AluOpType.mult,
                    accum_out=S[:, i : i + 1],
                )
            nc.vector.reduce_sum(
                S[:, DVE_BLOCKS:NBLK], prod[:], axis=mybir.AxisListType.X
            )
            # mask the query's self-score (t = 4095 -> p=127, i=31)
            nc.vector.tensor_add(
                S[:, NBLK - 1 : NBLK], S[:, NBLK - 1 : NBLK], maskbias[:]
            )

            rowmax = stat.tile([P, 1], F32)
            nc.vector.reduce_max(rowmax[:], S[:], axis=mybir.AxisListType.X)
            # cross-partition max: PE transpose -> free-dim max -> PE
            # broadcast back to all partitions with a -1 weight (fuses the
            # negation needed for the exp bias)
            rmT = statps.tile([1, P], F32, tag="sps")
            nc.tensor.transpose(rmT[:], rowmax[:], identity[:])
            gmax = stat.tile([1, 1], F32)
            nc.vector.reduce_max(gmax[:], rmT[:], axis=mybir.AxisListType.X)
            negb = statps.tile([P, 1], F32, tag="sps")
            nc.tensor.matmul(
                negb[:], lhsT=neg_ones_row[:], rhs=gmax[:], start=True, stop=True
            )
            negmax = stat.tile([P, 1], F32)
            nc.scalar.copy(negmax[:], negb[:])

            Pw = spool.tile([P, NBLK], FP16)
            rowsum = stat.tile([P, 1], F32)
            nc.scalar.activation(
                Pw[:],
                S[:],
                mybir.ActivationFunctionType.Exp,
                bias=negmax[:],
                scale=1.0,
                accum_out=rowsum[:],
            )

            Zp = statps.tile([1, 1], F32, tag="sps")
            nc.tensor.matmul(Zp[:], lhsT=rowsum[:], rhs=ones[:], start=True, stop=True)

            att = pspool.tile([1, D], F32)
            for i in range(NBLK):
                nc.tensor.matmul(
                    att[:],
                    lhsT=Pw[:, i : i + 1],
                    rhs=Xh[:, i, :],
                    start=(i == 0),
                    stop=(i == NBLK - 1),
                )

            rz = stat.tile([1, 1], F32)
            nc.vector.reciprocal(rz[:], Zp[:])
            att_sb = opool.tile([1, D], F32)
            nc.scalar.mul(att_sb[:], att[:], rz[:])

            nc.sync.dma_start(oap[b : b + 1, 0:D], F[0:1, :])
            nc.sync.dma_start(oap[b : b + 1, D : 2 * D], att_sb[:])

    nc.compile()
    return nc


def _run(x, trace=False):
    global _NC_CACHE
    x = np.ascontiguousarray(np.asarray(x, dtype=np.float32))
    assert x.shape == (B, T, D), x.shape
    if _NC_CACHE is None:
        _NC_CACHE = _build()
    in_maps = [{"x": x[c * BPC : (c + 1) * BPC]} for c in range(N_CORES)]
    res = run_bass_kernel_spmd(
        _NC_CACHE, in_maps, core_ids=list(range(N_CORES)), trace=trace
    )
    out = np.concatenate([res.results[c]["out"] for c in range(N_CORES)], axis=0)
    return out.astype(np.float32), res


def kernel(x):
    out, _ = _run(x, trace=False)
    return out


# revision 27
# speedup vs baseline: 1.3222x; 1.3222x over previous
"""Last-query sparse attention on 8 TRN2 NeuronCores.

Reference computation (per sample b):
    prev  = x[b, :-1, :]                 # [T-1, D]
    final = x[b, -1, :]                  # [D]
    s     = prev @ final                 # [T-1]
    w     = softmax(s)
    att   = w @ prev                     # [D]
    out   = concat(final, att)           # [2D]

Sharding: batch (B=64) split 8 ways -> 8 samples per core, no collectives.

Per-core layout: x[b] ([4096, 256] f32) is DMA'd to SBUF as [128, 32, 256]
with partition p holding rows t = p*32 + i (32KB contiguous HBM per
partition -> efficient descriptors).

Pass 1 (scores, contraction over free dim d): DVE tensor_tensor_reduce per
block i: accum[p] = sum_d x[p,i,d] * final[d] -> S[128, 32].
The self-score at t=4095 (p=127, i=31) is masked to -1e30.

Softmax: DVE row max -> GPSIMD partition_all_reduce(max) -> ACT exp with
per-partition bias (-gmax), fused row-sum accumulation; denominator via a
tiny PE matmul against a ones vector.

Pass 2 (weighted sum, contraction over t on partitions): PE matmuls
accumulating in PSUM: lhsT = exp-weights column [128, 1], rhs = x block
[128, 256] viewed as float32r (full-rate fp32 streaming), over 32 blocks.
Normalize by 1/Z on DVE, then DMA concat(final, att) to the output.
"""

import sys

sys.path.insert(0, "/opt/trn_rl_repo")

from contextlib import ExitStack

import numpy as np

import concourse.tile as tile
from concourse import bacc, mybir
from concourse.bass_utils import run_bass_kernel_spmd
from concourse.masks import make_identity

N_CORES = 8
B = 64
T = 4096
D = 256
BPC = B // N_CORES  # samples per core
P = 128
NBLK = T // P  # 32 blocks; t = p*NBLK + i
F32 = mybir.dt.float32
FP16 = mybir.dt.float16

_NC_CACHE = None


def _build():
    nc = bacc.Bacc(
        trn_type="TRN2",
        target_bir_lowering=False,
        debug=False,
        num_devices=N_CORES,
    )
    x_ext = nc.declare_dram_parameter("x", [BPC, T, D], F32, isOutput=False)
    out_ext = nc.declare_dram_parameter("out", [BPC, 2 * D], F32, isOutput=True)
    xap = x_ext.ap()
    oap = out_ext.ap()

    with ExitStack() as ctx:
        tc = ctx.enter_context(tile.TileContext(nc))
        xpool = ctx.enter_context(tc.tile_pool(name="xp", bufs=2))
        xbpool = ctx.enter_context(tc.tile_pool(name="xbp", bufs=2))
        fpool = ctx.enter_context(tc.tile_pool(name="fp", bufs=3))
        scrpool = ctx.enter_context(tc.tile_pool(name="scr", bufs=2))
        spool = ctx.enter_context(tc.tile_pool(name="sp", bufs=2))
        stat = ctx.enter_context(tc.tile_pool(name="stat", bufs=4))
        cpool = ctx.enter_context(tc.tile_pool(name="const", bufs=1))
        opool = ctx.enter_context(tc.tile_pool(name="outp", bufs=2))
        pspool = ctx.enter_context(tc.tile_pool(name="ps", bufs=2, space="PSUM"))
        statps = ctx.enter_context(tc.tile_pool(name="sps", bufs=6, space="PSUM"))

        ones = cpool.tile([P, 1], F32)
        nc.gpsimd.memset(ones[:], 1.0)
        neg_ones_row = cpool.tile([1, P], F32)
        nc.gpsimd.memset(neg_ones_row[:], -1.0)
        identity = cpool.tile([P, P], F32)
        make_identity(nc, identity[:])

        # maskbias[p] = -1e30 if p == 127 else 0 (masks the query's
        # self-score without touching a partition-127-based AP)
        pidx = cpool.tile([P, 1], mybir.dt.int32)
        nc.gpsimd.iota(pidx[:], pattern=[[0, 1]], base=0, channel_multiplier=1)
        maskbias = cpool.tile([P, 1], F32)
        nc.vector.tensor_scalar(
            out=maskbias[:],
            in0=pidx[:],
            scalar1=126,
            scalar2=None,
            op0=mybir.AluOpType.is_gt,
        )
        nc.vector.tensor_scalar_mul(maskbias[:], maskbias[:], -1.0e30)

        for b in range(BPC):
            X = xpool.tile([P, NBLK, D], F32)
            nc.sync.dma_start(X[:], xap[b].rearrange("(p i) d -> p i d", p=P))
            F = fpool.tile([P, D], F32)
            nc.sync.dma_start(F[:], xap[b, T - 1].partition_broadcast(P))

            # fp16 copies: pass 1 runs DVE tensor_tensor at 2x on 16-bit
            # data, pass 2 streams fp16 through the PE at full rate. ACT is
            # otherwise idle so the conversions overlap with DMA/DVE.
            Xh = xbpool.tile([P, NBLK, D], FP16)
            nc.scalar.copy(Xh[:], X[:])
            Fh = fpool.tile([P, D], FP16)
            nc.scalar.copy(Fh[:], F[:])

            # Pass 1 in four big DVE ops (fp16 2x mode for the first three):
            # products, two pairwise tree-add levels, then a segmented f32
            # reduce of the remaining 64 elements per score.
            S = spool.tile([P, NBLK], F32)
            prod = scrpool.tile([P, NBLK, D], FP16)
            nc.vector.tensor_mul(
                prod[:], Xh[:], Fh[:].unsqueeze(1).broadcast_to((P, NBLK, D))
            )
            l1 = scrpool.tile([P, NBLK, D // 2], FP16)
            nc.vector.tensor_add(
                l1[:], prod[:, :, 0 : D // 2], prod[:, :, D // 2 : D]
            )
            l2 = scrpool.tile([P, NBLK, D // 4], FP16)
            nc.vector.tensor_add(l2[:], l1[:, :, 0 : D // 4], l1[:, :, D // 4 : D // 2])
            nc.vector.reduce_sum(S[:], l2[:], axis=mybir.AxisListType.X)
            # mask the query's self-score (t = 4095 -> p=127, i=31)
            nc.vector.tensor_add(
                S[:, NBLK - 1 : NBLK], S[:, NBLK - 1 : NBLK], maskbias[:]
            )

            rowmax = stat.tile([P, 1], F32)
            nc.vector.reduce_max(rowmax[:], S[:], axis=mybir.AxisListType.X)
            # cross-partition max: PE transpose -> free-dim max -> PE
            # broadcast back to all partitions with a -1 weight (fuses the
            # negation needed for the exp bias)
            rmT = statps.tile([1, P], F32, tag="sps")
            nc.tensor.transpose(rmT[:], rowmax[:], identity[:])
            gmax = stat.tile([1, 1], F32)
            nc.vector.reduce_max(gmax[:], rmT[:], axis=mybir.AxisListType.X)
            negb = statps.tile([P, 1], F32, tag="sps")
            nc.tensor.matmul(
                negb[:], lhsT=neg_ones_row[:], rhs=gmax[:], start=True, stop=True
            )
            negmax = stat.tile([P, 1], F32)
            nc.scalar.copy(negmax[:], negb[:])

            Pw = spool.tile([P, NBLK], FP16)
            rowsum = stat.tile([P, 1], F32)
            nc.scalar.activation(
                Pw[:],
                S[:],
                mybir.ActivationFunctionType.Exp,
                bias=negmax[:],
                scale=1.0,
                accum_out=rowsum[:],
            )

            Zp = statps.tile([1, 1], F32, tag="sps")
            nc.tensor.matmul(Zp[:], lhsT=rowsum[:], rhs=ones[:], start=True, stop=True)

            att = pspool.tile([1, D], F32)
            for i in range(NBLK):
                nc.tensor.matmul(
                    att[:],
                    lhsT=Pw[:, i : i + 1],
                    rhs=Xh[:, i, :],
                    start=(i == 0),
                    stop=(i == NBLK - 1),
                )

            rz = stat.tile([1, 1], F32)
            nc.vector.reciprocal(rz[:], Zp[:])
            att_sb = opool.tile([1, D], F32)
            nc.scalar.mul(att_sb[:], att[:], rz[:])

            nc.sync.dma_start(oap[b : b + 1, 0:D], F[0:1, :])
            nc.sync.dma_start(oap[b : b + 1, D : 2 * D], att_sb[:])

    nc.compile()
    return nc


def _run(x, trace=False):
    global _NC_CACHE
    x = np.ascontiguousarray(np.asarray(x, dtype=np.float32))
    assert x.shape == (B, T, D), x.shape
    if _NC_CACHE is None:
        _NC_CACHE = _build()
    in_maps = [{"x": x[c * BPC : (c + 1) * BPC]} for c in range(N_CORES)]
    res = run_bass_kernel_spmd(
        _NC_CACHE, in_maps, core_ids=list(range(N_CORES)), trace=trace
    )
    out = np.concatenate([res.results[c]["out"] for c in range(N_CORES)], axis=0)
    return out.astype(np.float32), res


def kernel(x):
    out, _ = _run(x, trace=False)
    return out


# revision 29
# speedup vs baseline: 1.5783x; 1.1937x over previous
"""Last-query sparse attention on 8 TRN2 NeuronCores.

Reference computation (per sample b):
    prev  = x[b, :-1, :]                 # [T-1, D]
    final = x[b, -1, :]                  # [D]
    s     = prev @ final                 # [T-1]
    w     = softmax(s)
    att   = w @ prev                     # [D]
    out   = concat(final, att)           # [2D]

Sharding: batch (B=64) split 8 ways -> 8 samples per core, no collectives.

Per-core layout: x[b] ([4096, 256] f32) is DMA'd to SBUF as [128, 32, 256]
with partition p holding rows t = p*32 + i (32KB contiguous HBM per
partition -> efficient descriptors).

Pass 1 (scores, contraction over free dim d): DVE tensor_tensor_reduce per
block i: accum[p] = sum_d x[p,i,d] * final[d] -> S[128, 32].
The self-score at t=4095 (p=127, i=31) is masked to -1e30.

Softmax: DVE row max -> GPSIMD partition_all_reduce(max) -> ACT exp with
per-partition bias (-gmax), fused row-sum accumulation; denominator via a
tiny PE matmul against a ones vector.

Pass 2 (weighted sum, contraction over t on partitions): PE matmuls
accumulating in PSUM: lhsT = exp-weights column [128, 1], rhs = x block
[128, 256] viewed as float32r (full-rate fp32 streaming), over 32 blocks.
Normalize by 1/Z on DVE, then DMA concat(final, att) to the output.
"""

import sys

sys.path.insert(0, "/opt/trn_rl_repo")

from contextlib import ExitStack

import numpy as np

import concourse.tile as tile
from concourse import bacc, mybir
from concourse.bass_utils import run_bass_kernel_spmd
from concourse.masks import make_identity

N_CORES = 8
B = 64
T = 4096
D = 256
BPC = B // N_CORES  # samples per core
P = 128
NBLK = T // P  # 32 blocks; t = p*NBLK + i
F32 = mybir.dt.float32
FP16 = mybir.dt.float16

_NC_CACHE = None


def _build():
    nc = bacc.Bacc(
        trn_type="TRN2",
        target_bir_lowering=False,
        debug=False,
        num_devices=N_CORES,
    )
    x_ext = nc.declare_dram_parameter("x", [BPC, T, D], F32, isOutput=False)
    out_ext = nc.declare_dram_parameter("out", [BPC, 2 * D], F32, isOutput=True)
    xap = x_ext.ap()
    oap = out_ext.ap()

    with ExitStack() as ctx:
        tc = ctx.enter_context(tile.TileContext(nc))
        xbpool = ctx.enter_context(tc.tile_pool(name="xbp", bufs=4))
        fpool = ctx.enter_context(tc.tile_pool(name="fp", bufs=4))
        scrpool = ctx.enter_context(tc.tile_pool(name="scr", bufs=2))
        spool = ctx.enter_context(tc.tile_pool(name="sp", bufs=3))
        stat = ctx.enter_context(tc.tile_pool(name="stat", bufs=6))
        cpool = ctx.enter_context(tc.tile_pool(name="const", bufs=1))
        opool = ctx.enter_context(tc.tile_pool(name="outp", bufs=2))
        pspool = ctx.enter_context(tc.tile_pool(name="ps", bufs=2, space="PSUM"))
        statps = ctx.enter_context(tc.tile_pool(name="sps", bufs=6, space="PSUM"))

        ones = cpool.tile([P, 1], F32)
        nc.gpsimd.memset(ones[:], 1.0)
        neg_ones_row = cpool.tile([1, P], F32)
        nc.gpsimd.memset(neg_ones_row[:], -1.0)
        identity = cpool.tile([P, P], F32)
        make_identity(nc, identity[:])

        # maskbias[p] = -1e30 if p == 127 else 0 (masks the query's
        # self-score without touching a partition-127-based AP)
        pidx = cpool.tile([P, 1], mybir.dt.int32)
        nc.gpsimd.iota(pidx[:], pattern=[[0, 1]], base=0, channel_multiplier=1)
        maskbias = cpool.tile([P, 1], F32)
        nc.vector.tensor_scalar(
            out=maskbias[:],
            in0=pidx[:],
            scalar1=126,
            scalar2=None,
            op0=mybir.AluOpType.is_gt,
        )
        nc.vector.tensor_scalar_mul(maskbias[:], maskbias[:], -1.0e30)

        for b in range(BPC):
            # fp16 arrives straight off the DMA (SWDGE casts f32->fp16
            # inline): pass 1 runs DVE tensor_tensor at 2x on 16-bit data,
            # pass 2 streams fp16 through the PE at full rate. fp16 scores
            # keep 11 mantissa bits -> softmax output good to ~2e-3.
            Xh = xbpool.tile([P, NBLK, D], FP16)
            nc.gpsimd.dma_start(Xh[:], xap[b].rearrange("(p i) d -> p i d", p=P))
            F = fpool.tile([P, D], F32)
            nc.sync.dma_start(F[:], xap[b, T - 1].partition_broadcast(P))
            Fh = fpool.tile([P, D], FP16)
            nc.scalar.copy(Fh[:], F[:])

            # Pass 1 in four big DVE ops (fp16 2x mode for the first three):
            # products, two pairwise tree-add levels, then a segmented f32
            # reduce of the remaining 64 elements per score.
            S = spool.tile([P, NBLK], F32)
            prod = scrpool.tile([P, NBLK, D], FP16)
            nc.vector.tensor_mul(
                prod[:], Xh[:], Fh[:].unsqueeze(1).broadcast_to((P, NBLK, D))
            )
            l1 = scrpool.tile([P, NBLK, D // 2], FP16)
            nc.vector.tensor_add(
                l1[:], prod[:, :, 0 : D // 2], prod[:, :, D // 2 : D]
            )
            l2 = scrpool.tile([P, NBLK, D // 4], FP16)
            nc.vector.tensor_add(l2[:], l1[:, :, 0 : D // 4], l1[:, :, D // 4 : D // 2])
            nc.vector.reduce_sum(S[:], l2[:], axis=mybir.AxisListType.X)
            # mask the query's self-score (t = 4095 -> p=127, i=31)
            nc.vector.tensor_add(
                S[:, NBLK - 1 : NBLK], S[:, NBLK - 1 : NBLK], maskbias[:]
            )

            rowmax = stat.tile([P, 1], F32)
            nc.vector.reduce_max(rowmax[:], S[:], axis=mybir.AxisListType.X)
            # cross-partition max: PE transpose -> free-dim max -> PE
            # broadcast back to all partitions with a -1 weight (fuses the
            # negation needed for the exp bias)
            rmT = statps.tile([1, P], F32, tag="sps")
            nc.tensor.transpose(rmT[:], rowmax[:], identity[:])
            gmax = stat.tile([1, 1], F32)
            nc.vector.reduce_max(gmax[:], rmT[:], axis=mybir.AxisListType.X)
            negb = statps.tile([P, 1], F32, tag="sps")
            nc.tensor.matmul(
                negb[:], lhsT=neg_ones_row[:], rhs=gmax[:], start=True, stop=True
            )
            negmax = stat.tile([P, 1], F32)
            nc.scalar.copy(negmax[:], negb[:])

            Pw = spool.tile([P, NBLK], FP16)
            rowsum = stat.tile([P, 1], F32)
            nc.scalar.activation(
                Pw[:],
                S[:],
                mybir.ActivationFunctionType.Exp,
                bias=negmax[:],
                scale=1.0,
                accum_out=rowsum[:],
            )

            Zp = statps.tile([1, 1], F32, tag="sps")
            nc.tensor.matmul(Zp[:], lhsT=rowsum[:], rhs=ones[:], start=True, stop=True)

            att = pspool.tile([1, D], F32)
            for i in range(NBLK):
                nc.tensor.matmul(
                    att[:],
                    lhsT=Pw[:, i : i + 1],
                    rhs=Xh[:, i, :],
                    start=(i == 0),
                    stop=(i == NBLK - 1),
                )

            rz = stat.tile([1, 1], F32)
            nc.vector.reciprocal(rz[:], Zp[:])
            att_sb = opool.tile([1, D], F32)
            nc.scalar.mul(att_sb[:], att[:], rz[:])

            nc.sync.dma_start(oap[b : b + 1, 0:D], F[0:1, :])
            nc.sync.dma_start(oap[b : b + 1, D : 2 * D], att_sb[:])

    nc.compile()
    return nc


def _run(x, trace=False):
    global _NC_CACHE
    x = np.ascontiguousarray(np.asarray(x, dtype=np.float32))
    assert x.shape == (B, T, D), x.shape
    if _NC_CACHE is None:
        _NC_CACHE = _build()
    in_maps = [{"x": x[c * BPC : (c + 1) * BPC]} for c in range(N_CORES)]
    res = run_bass_kernel_spmd(
        _NC_CACHE, in_maps, core_ids=list(range(N_CORES)), trace=trace
    )
    out = np.concatenate([res.results[c]["out"] for c in range(N_CORES)], axis=0)
    return out.astype(np.float32), res


def kernel(x):
    out, _ = _run(x, trace=False)
    return out


# revision 32
# speedup vs baseline: 1.6020x; 1.0150x over previous
"""Last-query sparse attention on 8 TRN2 NeuronCores.

Reference computation (per sample b):
    prev  = x[b, :-1, :]                 # [T-1, D]
    final = x[b, -1, :]                  # [D]
    s     = prev @ final                 # [T-1]
    w     = softmax(s)
    att   = w @ prev                     # [D]
    out   = concat(final, att)           # [2D]

Sharding: batch (B=64) split 8 ways -> 8 samples per core, no collectives.

Per-core layout: x[b] ([4096, 256] f32) is DMA'd to SBUF as [128, 32, 256]
with partition p holding rows t = p*32 + i (32KB contiguous HBM per
partition -> efficient descriptors).

Pass 1 (scores, contraction over free dim d): DVE tensor_tensor_reduce per
block i: accum[p] = sum_d x[p,i,d] * final[d] -> S[128, 32].
The self-score at t=4095 (p=127, i=31) is masked to -1e30.

Softmax: DVE row max -> GPSIMD partition_all_reduce(max) -> ACT exp with
per-partition bias (-gmax), fused row-sum accumulation; denominator via a
tiny PE matmul against a ones vector.

Pass 2 (weighted sum, contraction over t on partitions): PE matmuls
accumulating in PSUM: lhsT = exp-weights column [128, 1], rhs = x block
[128, 256] viewed as float32r (full-rate fp32 streaming), over 32 blocks.
Normalize by 1/Z on DVE, then DMA concat(final, att) to the output.
"""

import sys

sys.path.insert(0, "/opt/trn_rl_repo")

from contextlib import ExitStack

import numpy as np

import concourse.tile as tile
import concourse.bass_isa as bass_isa
from concourse import bacc, mybir
from concourse.bass_utils import run_bass_kernel_spmd

N_CORES = 8
B = 64
T = 4096
D = 256
BPC = B // N_CORES  # samples per core
P = 128
NBLK = T // P  # 32 blocks; t = p*NBLK + i
F32 = mybir.dt.float32
FP16 = mybir.dt.float16

_NC_CACHE = None


def _build():
    nc = bacc.Bacc(
        trn_type="TRN2",
        target_bir_lowering=False,
        debug=False,
        num_devices=N_CORES,
    )
    x_ext = nc.declare_dram_parameter("x", [BPC, T, D], F32, isOutput=False)
    out_ext = nc.declare_dram_parameter("out", [BPC, 2 * D], F32, isOutput=True)
    xap = x_ext.ap()
    oap = out_ext.ap()

    with ExitStack() as ctx:
        tc = ctx.enter_context(tile.TileContext(nc))
        xbpool = ctx.enter_context(tc.tile_pool(name="xbp", bufs=4))
        fpool = ctx.enter_context(tc.tile_pool(name="fp", bufs=4))
        scrpool = ctx.enter_context(tc.tile_pool(name="scr", bufs=2))
        spool = ctx.enter_context(tc.tile_pool(name="sp", bufs=3))
        stat = ctx.enter_context(tc.tile_pool(name="stat", bufs=6))
        cpool = ctx.enter_context(tc.tile_pool(name="const", bufs=1))
        opool = ctx.enter_context(tc.tile_pool(name="outp", bufs=2))
        pspool = ctx.enter_context(tc.tile_pool(name="ps", bufs=2, space="PSUM"))
        statps = ctx.enter_context(tc.tile_pool(name="sps", bufs=6, space="PSUM"))

        # maskbias[p] = -1e30 if p == 127 else 0 (masks the query's
        # self-score without touching a partition-127-based AP)
        pidx = cpool.tile([P, 1], mybir.dt.int32)
        nc.gpsimd.iota(pidx[:], pattern=[[0, 1]], base=0, channel_multiplier=1)
        maskbias = cpool.tile([P, 1], F32)
        nc.vector.tensor_scalar(
            out=maskbias[:],
            in0=pidx[:],
            scalar1=126,
            scalar2=None,
            op0=mybir.AluOpType.is_gt,
        )
        nc.vector.tensor_scalar_mul(maskbias[:], maskbias[:], -1.0e30)

        for b in range(BPC):
            # fp16 arrives straight off the DMA (SWDGE casts f32->fp16
            # inline): pass 1 runs DVE tensor_tensor at 2x on 16-bit data,
            # pass 2 streams fp16 through the PE at full rate. fp16 scores
            # keep 11 mantissa bits -> softmax output good to ~2e-3.
            Xh = xbpool.tile([P, NBLK, D], FP16)
            nc.gpsimd.dma_start(Xh[:], xap[b].rearrange("(p i) d -> p i d", p=P))
            F = fpool.tile([P, D], F32)
            nc.sync.dma_start(F[:], xap[b, T - 1].partition_broadcast(P))
            Fh = fpool.tile([P, D], FP16)
            nc.scalar.copy(Fh[:], F[:])

            # Pass 1 in four big DVE ops (fp16 2x mode for the first three):
            # products, two pairwise tree-add levels, then a segmented f32
            # reduce of the remaining 64 elements per score.
            S = spool.tile([P, NBLK], F32)
            prod = scrpool.tile([P, NBLK, D], FP16)
            nc.vector.tensor_mul(
                prod[:], Xh[:], Fh[:].unsqueeze(1).broadcast_to((P, NBLK, D))
            )
            l1 = scrpool.tile([P, NBLK, D // 2], FP16)
            nc.vector.tensor_add(
                l1[:], prod[:, :, 0 : D // 2], prod[:, :, D // 2 : D]
            )
            l2 = scrpool.tile([P, NBLK, D // 4], FP16)
            nc.vector.tensor_add(l2[:], l1[:, :, 0 : D // 4], l1[:, :, D // 4 : D // 2])
            nc.vector.reduce_sum(S[:], l2[:], axis=mybir.AxisListType.X)
            # mask the query's self-score (t = 4095 -> p=127, i=31)
            nc.vector.tensor_add(
                S[:, NBLK - 1 : NBLK], S[:, NBLK - 1 : NBLK], maskbias[:]
            )

            rowmax = stat.tile([P, 1], F32)
            nc.vector.reduce_max(rowmax[:], S[:], axis=mybir.AxisListType.X)
            # cross-partition max on GPSIMD (Q7 attn library), negate on ACT
            gmax = stat.tile([P, 1], F32)
            nc.gpsimd.partition_all_reduce(
                gmax[:], rowmax[:], channels=P, reduce_op=bass_isa.ReduceOp.max
            )
            negmax = stat.tile([P, 1], F32)
            nc.scalar.mul(negmax[:], gmax[:], -1.0)

            Pw = spool.tile([P, NBLK], FP16)
            rowsum = stat.tile([P, 1], F32)
            nc.scalar.activation(
                Pw[:],
                S[:],
                mybir.ActivationFunctionType.Exp,
                bias=negmax[:],
                scale=1.0,
                accum_out=rowsum[:],
            )

            # denominator: cross-partition sum of the exp row-sums
            Zp = stat.tile([P, 1], F32)
            nc.gpsimd.partition_all_reduce(
                Zp[:], rowsum[:], channels=P, reduce_op=bass_isa.ReduceOp.add
            )

            att = pspool.tile([1, D], F32)
            for i in range(NBLK):
                nc.tensor.matmul(
                    att[:],
                    lhsT=Pw[:, i : i + 1],
                    rhs=Xh[:, i, :],
                    start=(i == 0),
                    stop=(i == NBLK - 1),
                )

            rz = stat.tile([1, 1], F32)
            nc.vector.reciprocal(rz[:], Zp[0:1, 0:1])
            att_sb = opool.tile([1, D], F32)
            nc.scalar.mul(att_sb[:], att[:], rz[:])

            nc.sync.dma_start(oap[b : b + 1, 0:D], F[0:1, :])
            nc.sync.dma_start(oap[b : b + 1, D : 2 * D], att_sb[:])

    nc.compile()
    return nc


def _run(x, trace=False):
    global _NC_CACHE
    x = np.ascontiguousarray(np.asarray(x, dtype=np.float32))
    assert x.shape == (B, T, D), x.shape
    if _NC_CACHE is None:
        _NC_CACHE = _build()
    in_maps = [{"x": x[c * BPC : (c + 1) * BPC]} for c in range(N_CORES)]
    res = run_bass_kernel_spmd(
        _NC_CACHE, in_maps, core_ids=list(range(N_CORES)), trace=trace
    )
    out = np.concatenate([res.results[c]["out"] for c in range(N_CORES)], axis=0)
    return out.astype(np.float32), res


def kernel(x):
    out, _ = _run(x, trace=False)
    return out


# revision 34
# speedup vs baseline: 1.8011x; 1.1243x over previous
"""Last-query sparse attention on 8 TRN2 NeuronCores.

Reference computation (per sample b):
    prev  = x[b, :-1, :]                 # [T-1, D]
    final = x[b, -1, :]                  # [D]
    s     = prev @ final                 # [T-1]
    w     = softmax(s)
    att   = w @ prev                     # [D]
    out   = concat(final, att)           # [2D]

Sharding: batch (B=64) split 8 ways -> 8 samples per core, no collectives.

Per-core layout: x[b] ([4096, 256] f32) is DMA'd to SBUF as [128, 32, 256]
with partition p holding rows t = p*32 + i (32KB contiguous HBM per
partition -> efficient descriptors).

Pass 1 (scores, contraction over free dim d): DVE tensor_tensor_reduce per
block i: accum[p] = sum_d x[p,i,d] * final[d] -> S[128, 32].
The self-score at t=4095 (p=127, i=31) is masked to -1e30.

Softmax: DVE row max -> GPSIMD partition_all_reduce(max) -> ACT exp with
per-partition bias (-gmax), fused row-sum accumulation; denominator via a
tiny PE matmul against a ones vector.

Pass 2 (weighted sum, contraction over t on partitions): PE matmuls
accumulating in PSUM: lhsT = exp-weights column [128, 1], rhs = x block
[128, 256] viewed as float32r (full-rate fp32 streaming), over 32 blocks.
Normalize by 1/Z on DVE, then DMA concat(final, att) to the output.
"""

import sys

sys.path.insert(0, "/opt/trn_rl_repo")

from contextlib import ExitStack

import numpy as np

import concourse.tile as tile
import concourse.bass_isa as bass_isa
from concourse import bacc, mybir
from concourse.bass_utils import run_bass_kernel_spmd

N_CORES = 8
B = 64
T = 4096
D = 256
BPC = B // N_CORES  # samples per core
P = 128
NBLK = T // P  # 32 blocks; t = p*NBLK + i
F32 = mybir.dt.float32
FP16 = mybir.dt.float16

_NC_CACHE = None


def _build():
    nc = bacc.Bacc(
        trn_type="TRN2",
        target_bir_lowering=False,
        debug=False,
        num_devices=N_CORES,
    )
    x_ext = nc.declare_dram_parameter("x", [BPC, T, D], F32, isOutput=False)
    out_ext = nc.declare_dram_parameter("out", [BPC, 2 * D], F32, isOutput=True)
    xap = x_ext.ap()
    oap = out_ext.ap()

    with ExitStack() as ctx:
        tc = ctx.enter_context(tile.TileContext(nc))
        xbpool = ctx.enter_context(tc.tile_pool(name="xbp", bufs=6))
        fpool = ctx.enter_context(tc.tile_pool(name="fp", bufs=4))
        scrpool = ctx.enter_context(tc.tile_pool(name="scr", bufs=3))
        spool = ctx.enter_context(tc.tile_pool(name="sp", bufs=3))
        stat = ctx.enter_context(tc.tile_pool(name="stat", bufs=6))
        cpool = ctx.enter_context(tc.tile_pool(name="const", bufs=1))
        opool = ctx.enter_context(tc.tile_pool(name="outp", bufs=2))
        pspool = ctx.enter_context(tc.tile_pool(name="ps", bufs=2, space="PSUM"))
        statps = ctx.enter_context(tc.tile_pool(name="sps", bufs=6, space="PSUM"))

        # maskbias[p] = -1e30 if p == 127 else 0 (masks the query's
        # self-score without touching a partition-127-based AP)
        pidx = cpool.tile([P, 1], mybir.dt.int32)
        nc.gpsimd.iota(pidx[:], pattern=[[0, 1]], base=0, channel_multiplier=1)
        maskbias = cpool.tile([P, 1], F32)
        nc.vector.tensor_scalar(
            out=maskbias[:],
            in0=pidx[:],
            scalar1=126,
            scalar2=None,
            op0=mybir.AluOpType.is_gt,
        )
        nc.vector.tensor_scalar_mul(maskbias[:], maskbias[:], -1.0e30)

        HB = NBLK // 2  # half-sample granularity for DMA/pass-1 pipelining
        for b in range(BPC):
            # fp16 arrives straight off the DMA (SWDGE casts f32->fp16
            # inline): pass 1 runs DVE tensor_tensor at 2x on 16-bit data,
            # pass 2 streams fp16 through the PE at full rate. fp16 scores
            # keep 11 mantissa bits -> softmax output good to ~2e-3.
            # The load and pass 1 are split into two halves so compute can
            # start as soon as the first 2MB lands.
            Xh = xbpool.tile([P, NBLK, D], FP16)
            xr = xap[b].rearrange("(p i) d -> p i d", p=P)
            nc.gpsimd.dma_start(Xh[:, 0:HB, :], xr[:, 0:HB, :])
            F = fpool.tile([P, D], F32)
            nc.sync.dma_start(F[:], xap[b, T - 1].partition_broadcast(P))
            Fh = fpool.tile([P, D], FP16)
            nc.scalar.copy(Fh[:], F[:])
            nc.gpsimd.dma_start(Xh[:, HB:NBLK, :], xr[:, HB:NBLK, :])

            # Pass 1 per half in four big DVE ops (fp16 2x mode for the
            # first three): products, two pairwise tree-add levels, then a
            # segmented f32 reduce of the remaining 64 elements per score.
            S = spool.tile([P, NBLK], F32)
            for h in range(2):
                blo, bhi = h * HB, (h + 1) * HB
                prod = scrpool.tile([P, HB, D], FP16, tag="prod")
                nc.vector.tensor_mul(
                    prod[:],
                    Xh[:, blo:bhi, :],
                    Fh[:].unsqueeze(1).broadcast_to((P, HB, D)),
                )
                l1 = scrpool.tile([P, HB, D // 2], FP16, tag="l1")
                nc.vector.tensor_add(
                    l1[:], prod[:, :, 0 : D // 2], prod[:, :, D // 2 : D]
                )
                l2 = scrpool.tile([P, HB, D // 4], FP16, tag="l2")
                nc.vector.tensor_add(
                    l2[:], l1[:, :, 0 : D // 4], l1[:, :, D // 4 : D // 2]
                )
                nc.vector.reduce_sum(S[:, blo:bhi], l2[:], axis=mybir.AxisListType.X)
            # mask the query's self-score (t = 4095 -> p=127, i=31)
            nc.vector.tensor_add(
                S[:, NBLK - 1 : NBLK], S[:, NBLK - 1 : NBLK], maskbias[:]
            )

            rowmax = stat.tile([P, 1], F32)
            nc.vector.reduce_max(rowmax[:], S[:], axis=mybir.AxisListType.X)
            # cross-partition max on GPSIMD (Q7 attn library), negate on ACT
            gmax = stat.tile([P, 1], F32)
            nc.gpsimd.partition_all_reduce(
                gmax[:], rowmax[:], channels=P, reduce_op=bass_isa.ReduceOp.max
            )
            negmax = stat.tile([P, 1], F32)
            nc.scalar.mul(negmax[:], gmax[:], -1.0)

            Pw = spool.tile([P, NBLK], FP16)
            rowsum = stat.tile([P, 1], F32)
            nc.scalar.activation(
                Pw[:],
                S[:],
                mybir.ActivationFunctionType.Exp,
                bias=negmax[:],
                scale=1.0,
                accum_out=rowsum[:],
            )

            # denominator: cross-partition sum of the exp row-sums
            Zp = stat.tile([P, 1], F32)
            nc.gpsimd.partition_all_reduce(
                Zp[:], rowsum[:], channels=P, reduce_op=bass_isa.ReduceOp.add
            )

            att = pspool.tile([1, D], F32)
            for i in range(NBLK):
                nc.tensor.matmul(
                    att[:],
                    lhsT=Pw[:, i : i + 1],
                    rhs=Xh[:, i, :],
                    start=(i == 0),
                    stop=(i == NBLK - 1),
                )

            rz = stat.tile([1, 1], F32)
            nc.vector.reciprocal(rz[:], Zp[0:1, 0:1])
            att_sb = opool.tile([1, D], F32)
            nc.scalar.mul(att_sb[:], att[:], rz[:])

            nc.sync.dma_start(oap[b : b + 1, 0:D], F[0:1, :])
            nc.sync.dma_start(oap[b : b + 1, D : 2 * D], att_sb[:])

    nc.compile()
    return nc


def _run(x, trace=False):
    global _NC_CACHE
    x = np.ascontiguousarray(np.asarray(x, dtype=np.float32))
    assert x.shape == (B, T, D), x.shape
    if _NC_CACHE is None:
        _NC_CACHE = _build()
    in_maps = [{"x": x[c * BPC : (c + 1) * BPC]} for c in range(N_CORES)]
    res = run_bass_kernel_spmd(
        _NC_CACHE, in_maps, core_ids=list(range(N_CORES)), trace=trace
    )
    out = np.concatenate([res.results[c]["out"] for c in range(N_CORES)], axis=0)
    return out.astype(np.float32), res


def kernel(x):
    out, _ = _run(x, trace=False)
    return out
